# revision 6
# baseline (speedup 1.0000x reference)
"""AttentiveFPConv GNN message-passing kernel for 8 Trainium2 NeuronCores.

Reference computation (all fp32):
    alpha = sigmoid(x[col] @ Wa_w + Wa_b)          # per-edge attention
    neigh = x[col] * alpha                          # per-edge message
    aggr  = segment_sum(neigh, row, N)              # per-node aggregation
    out   = tanh(x @ Wn_w + Wn_b + aggr @ Wg_w + Wg_b)

Key algebraic identity: alpha depends only on the source node, so
    h = x * sigmoid(x @ Wa_w + Wa_b)                # per-NODE tensor
    aggr[n] = sum_{e: row[e]=n} h[col[e]]           # gather + segment-sum

Sharding: destination-node sharding. Core k owns nodes [5000k, 5000(k+1))
and ALL edges targeting them (balanced: rows are uniform). No collective
needed: each core computes its own aggr and output slice.

Per-core pipeline:
  Phase 1: h = x*sigmoid(x@Wa+b) for ALL nodes (replicated), h -> HBM bf16.
  Phase 2: dma_gather h[col] in destination-sorted edge order; segment-sum
           via one-hot matmuls accumulating aggr^T in PSUM per 128-node block.
           (dma_gather indices are int16, so edges are split into two streams
           by col < 32768, each gathered against a rebased HBM view.)
  Phase 3: out = tanh(x@Wn + aggr@Wg + bias) per 128-node block.
"""

import numpy as np
import ml_dtypes

BF16 = ml_dtypes.bfloat16

# ---------------------------------------------------------------- parameters

class P:
    """Problem/kernel parameters (full-size defaults; shrinkable for tests)."""
    def __init__(self, N=40000, D=128, NCORES=8, HSPLIT=32768,
                 GCHUNK=1024, PH1_CHUNK=2048):
        assert D == 128
        self.N, self.D, self.NCORES = N, D, NCORES
        self.NB = N // NCORES                 # nodes per core
        self.HSPLIT = HSPLIT                  # col split for int16 gather idx
        self.GCHUNK = GCHUNK                  # idxs per dma_gather (HW limit ~1024)
        self.GT = GCHUNK // 128               # gather tiles per chunk
        self.PH1_CHUNK = PH1_CHUNK            # nodes per phase-1 xT chunk
        self.NBLK = (self.NB + 127) // 128    # 128-node blocks per core


# ------------------------------------------------------------ host edge prep

def prep_edges(p: P, row: np.ndarray, col: np.ndarray):
    """Per-core destination-sorted, block-padded edge streams.

    Returns (tA, tB, per_core) where tA/tB are per-block tile counts
    (identical across cores; max over cores) and per_core[k] is a dict with
    int16 gather index planes and bf16 local-row planes for streams A and B.
    """
    row = np.asarray(row).astype(np.int64)
    col = np.asarray(col).astype(np.int64)
    cores = []
    for k in range(p.NCORES):
        sel = (row // p.NB) == k
        r = (row[sel] - k * p.NB).astype(np.int32)
        c = col[sel].astype(np.int32)
        order = np.argsort(r, kind="stable")
        r, c = r[order], c[order]
        # block boundaries
        lo = np.searchsorted(r, np.arange(p.NBLK) * 128)
        hi = np.searchsorted(r, np.minimum(np.arange(1, p.NBLK + 1) * 128, p.NB))
        blocks = []
        for b in range(p.NBLK):
            rb = r[lo[b]:hi[b]] - b * 128
            cb = c[lo[b]:hi[b]]
            mA = cb < p.HSPLIT
            blocks.append(((cb[mA], rb[mA]), (cb[~mA] - p.HSPLIT, rb[~mA])))
        cores.append(blocks)

    nA = np.array([[len(cores[k][b][0][0]) for b in range(p.NBLK)]
                   for k in range(p.NCORES)])
    nB = np.array([[len(cores[k][b][1][0]) for b in range(p.NBLK)]
                   for k in range(p.NCORES)])
    tA = np.maximum(1, -(-nA.max(axis=0) // 128))          # [NBLK]
    tB = np.maximum(1, -(-nB.max(axis=0) // 128))

    LA, LB = int(tA.sum()) * 128, int(tB.sum()) * 128
    LAg = -(-LA // p.GCHUNK) * p.GCHUNK
    LBg = -(-LB // p.GCHUNK) * p.GCHUNK

    per_core = []
    for k in range(p.NCORES):
        idxA = np.zeros(LAg, np.int16); lrA = np.full(LA, -1.0, np.float32)
        idxB = np.zeros(LBg, np.int16); lrB = np.full(LB, -1.0, np.float32)
        oA = oB = 0
        for b in range(p.NBLK):
            (cA, rA), (cB, rB) = cores[k][b]
            idxA[oA:oA + len(cA)] = cA; lrA[oA:oA + len(rA)] = rA
            oA += int(tA[b]) * 128
            idxB[oB:oB + len(cB)] = cB; lrB[oB:oB + len(rB)] = rB
            oB += int(tB[b]) * 128
        per_core.append({
            "idxA": np.tile(idxA.reshape(-1, 16).T, (8, 1)),   # [128, LAg/16]
            "idxB": np.tile(idxB.reshape(-1, 16).T, (8, 1)),
            "lrA": lrA.reshape(-1, 128).T.copy(),              # [128, LA/128]
            "lrB": lrB.reshape(-1, 128).T.copy(),
        })
    return tA, tB, LA, LB, LAg, LBg, per_core


# ------------------------------------------------------------- device kernel

def build(p: P, tA, tB, LA, LB, LAg, LBg):
    from concourse import bacc, mybir, tile

    f32, bf16, i16 = mybir.dt.float32, mybir.dt.bfloat16, mybir.dt.int16
    nc = bacc.Bacc("TRN2", target_bir_lowering=False, debug=False,
                   num_devices=p.NCORES)

    N, D, NB, NBLK = p.N, p.D, p.NB, p.NBLK

    xT_d   = nc.dram_tensor("xT", [D, N], bf16, kind="ExternalInput")
    xTo_d  = nc.dram_tensor("xT_own", [D, NB], bf16, kind="ExternalInput")
    WaW_d  = nc.dram_tensor("WaW", [D, D], bf16, kind="ExternalInput")
    WaB_d  = nc.dram_tensor("WaB", [D, 1], f32, kind="ExternalInput")
    WnW_d  = nc.dram_tensor("WnW", [D, D], bf16, kind="ExternalInput")
    WgW_d  = nc.dram_tensor("WgW", [D, D], bf16, kind="ExternalInput")
    bias_d = nc.dram_tensor("biasNG", [D, D], f32, kind="ExternalInput")
    iota_d = nc.dram_tensor("iota", [D, D], bf16, kind="ExternalInput")
    ident_d= nc.dram_tensor("ident", [D, D], bf16, kind="ExternalInput")
    idxA_d = nc.dram_tensor("idxA", [128, LAg // 16], i16, kind="ExternalInput")
    idxB_d = nc.dram_tensor("idxB", [128, LBg // 16], i16, kind="ExternalInput")
    lrA_d  = nc.dram_tensor("lrA", [128, LA // 128], f32, kind="ExternalInput")
    lrB_d  = nc.dram_tensor("lrB", [128, LB // 128], f32, kind="ExternalInput")
    out_d  = nc.dram_tensor("out", [NB, D], f32, kind="ExternalOutput")
    h_d    = nc.dram_tensor("h", [N, D], bf16, kind="Internal")

    with tile.TileContext(nc) as tc:
        with (
            tc.tile_pool(name="const", bufs=1) as cpool,
            tc.tile_pool(name="xchunk", bufs=3) as xpool,
            tc.tile_pool(name="hstage", bufs=2) as hspool,
            tc.tile_pool(name="pg", bufs=2, space="PSUM") as pg_pool,
            tc.tile_pool(name="pt", bufs=2, space="PSUM") as pt_pool,
            tc.tile_pool(name="pa", bufs=2, space="PSUM") as pa_pool,
            tc.tile_pool(name="po", bufs=2, space="PSUM") as po_pool,
            tc.tile_pool(name="sA", bufs=16) as gApool,
            tc.tile_pool(name="sB", bufs=8) as gBpool,
            tc.tile_pool(name="m", bufs=4) as mpool,
            tc.tile_pool(name="agg", bufs=4) as aggpool,
            tc.tile_pool(name="ph1w", bufs=3) as w1pool,
            tc.tile_pool(name="ostage", bufs=2) as ospool,
        ):
            # ---- constants into SBUF
            WaW = cpool.tile([D, D], bf16); nc.sync.dma_start(out=WaW[:], in_=WaW_d[:])
            WaB = cpool.tile([D, 1], f32); nc.sync.dma_start(out=WaB[:], in_=WaB_d[:])
            WnW = cpool.tile([D, D], bf16); nc.sync.dma_start(out=WnW[:], in_=WnW_d[:])
            WgW = cpool.tile([D, D], bf16); nc.sync.dma_start(out=WgW[:], in_=WgW_d[:])
            bias = cpool.tile([D, D], f32); nc.sync.dma_start(out=bias[:], in_=bias_d[:])
            iota = cpool.tile([D, D], bf16); nc.sync.dma_start(out=iota[:], in_=iota_d[:])
            ident = cpool.tile([D, D], bf16); nc.sync.dma_start(out=ident[:], in_=ident_d[:])
            xT_own = cpool.tile([D, NB], bf16); nc.sync.dma_start(out=xT_own[:], in_=xTo_d[:])
            idxA_sb = cpool.tile([128, LAg // 16], i16)
            nc.sync.dma_start(out=idxA_sb[:], in_=idxA_d[:])
            idxB_sb = cpool.tile([128, LBg // 16], i16)
            nc.sync.dma_start(out=idxB_sb[:], in_=idxB_d[:])
            lrA_sb = cpool.tile([128, LA // 128], f32)
            nc.sync.dma_start(out=lrA_sb[:], in_=lrA_d[:])
            lrB_sb = cpool.tile([128, LB // 128], f32)
            nc.sync.dma_start(out=lrB_sb[:], in_=lrB_d[:])

            # ---- phase 1: h = x * sigmoid(x@Wa + b), all N nodes, h -> HBM
            base = 0
            while base < N:
                cn = min(p.PH1_CHUNK, N - base)       # nodes in this chunk
                nfull = cn // 128
                rem = cn - nfull * 128
                xc = xpool.tile([D, p.PH1_CHUNK], bf16, tag="xc")
                nc.sync.dma_start(out=xc[:, :cn], in_=xT_d[:, base:base + cn])
                hst = hspool.tile([128, p.PH1_CHUNK], bf16, tag="hst")
                nblk_local = nfull + (1 if rem else 0)
                for t in range(nblk_local):
                    nb = 128 if t < nfull else rem
                    co = t * 128
                    pg = pg_pool.tile([D, 128], f32, tag="pg")
                    nc.tensor.matmul(pg[:, :nb], lhsT=WaW[:], rhs=xc[:, co:co + nb],
                                     start=True, stop=True)
                    sT = w1pool.tile([D, 128], bf16, tag="sT")
                    nc.scalar.activation(sT[:, :nb], pg[:, :nb],
                                         mybir.ActivationFunctionType.Sigmoid,
                                         bias=WaB[:, 0:1])
                    hT = w1pool.tile([D, 128], bf16, tag="hT")
                    nc.vector.tensor_tensor(out=hT[:, :nb], in0=xc[:, co:co + nb],
                                            in1=sT[:, :nb], op=mybir.AluOpType.mult)
                    pt = pt_pool.tile([128, 128], bf16, tag="pt")
                    nc.tensor.transpose(pt[:nb, :], hT[:, :nb], ident[:])
                    nc.vector.tensor_copy(out=hst[:nb, co:co + 128], in_=pt[:nb, :])
                if nfull:
                    nc.sync.dma_start(
                        out=h_d[base:base + nfull * 128, :].rearrange(
                            "(t p) d -> p t d", p=128),
                        in_=hst[:, :nfull * 128].rearrange(
                            "p (t d) -> p t d", d=128))
                if rem:
                    nc.sync.dma_start(
                        out=h_d[base + nfull * 128:base + cn, :],
                        in_=hst[:rem, nfull * 128:nfull * 128 + 128])
                base += cn

            # ---- phases 2+3 per 128-node output block
            nA_chunks = LAg // p.GCHUNK
            nB_chunks = LBg // p.GCHUNK
            gA_tiles = [None] * nA_chunks
            gB_tiles = [None] * nB_chunks

            def ensure_chunk(tiles, which, ci):
                if tiles[ci] is not None:
                    return
                g = (gApool if which == "A" else gBpool).tile(
                    [128, p.GT, D], bf16, tag="g" + which)
                idx_sb = idxA_sb if which == "A" else idxB_sb
                src = h_d[:, :] if which == "A" else h_d[p.HSPLIT:, :]
                c0 = ci * (p.GCHUNK // 16)
                nc.gpsimd.dma_gather(
                    out_ap=g[:], in_ap=src, idxs_ap=idx_sb[:, c0:c0 + p.GCHUNK // 16],
                    num_idxs=p.GCHUNK, num_idxs_reg=p.GCHUNK, elem_size=D)
                tiles[ci] = g

            posA = np.concatenate([[0], np.cumsum(tA)]).astype(int)
            posB = np.concatenate([[0], np.cumsum(tB)]).astype(int)
            OCH = 8  # output blocks per staged DMA
            ost = None
            ost_base = 0
            ost_n = 0
            for b in range(NBLK):
                nb = min(128, NB - b * 128)
                pa = pa_pool.tile([D, 128], f32, tag="pa")
                ntiles = int(tA[b]) + int(tB[b])
                ti = 0
                for which, tcnt, pos, lr_sb, tiles in (
                        ("A", int(tA[b]), posA, lrA_sb, gA_tiles),
                        ("B", int(tB[b]), posB, lrB_sb, gB_tiles)):
                    for j in range(tcnt):
                        g = pos[b] + j
                        ensure_chunk(tiles, which, g // p.GT)
                        neigh = tiles[g // p.GT][:, g % p.GT, :]
                        M = mpool.tile([128, 128], bf16, tag="M")
                        nc.vector.tensor_scalar(
                            out=M[:], in0=iota[:], scalar1=lr_sb[:, g:g + 1],
                            scalar2=None, op0=mybir.AluOpType.is_equal)
                        nc.tensor.matmul(pa[:, :], lhsT=neigh, rhs=M[:],
                                         start=(ti == 0), stop=(ti == ntiles - 1))
                        ti += 1
                aggT = aggpool.tile([D, 128], bf16, tag="aggT")
                nc.vector.tensor_copy(out=aggT[:], in_=pa[:])

                # phase 3 for this block
                po = po_pool.tile([128, D], f32, tag="po")
                nc.tensor.matmul(po[:nb, :], lhsT=xT_own[:, b * 128:b * 128 + nb],
                                 rhs=WnW[:], start=True, stop=False)
                nc.tensor.matmul(po[:nb, :], lhsT=aggT[:, :nb], rhs=WgW[:],
                                 start=False, stop=True)
                nc.vector.tensor_tensor(out=po[:nb, :], in0=po[:nb, :],
                                        in1=bias[:nb, :], op=mybir.AluOpType.add)
                if ost is None:
                    ost = ospool.tile([128, OCH * D], f32, tag="ost")
                    ost_base = b
                    ost_n = 0
                nc.scalar.activation(ost[:nb, ost_n * D:(ost_n + 1) * D], po[:nb, :],
                                     mybir.ActivationFunctionType.Tanh)
                ost_n += 1
                flush = (ost_n == OCH) or (b == NBLK - 1)
                if flush:
                    rows0 = ost_base * 128
                    nfull_o = ost_n if nb == 128 else ost_n - 1
                    if nfull_o:
                        nc.sync.dma_start(
                            out=out_d[rows0:rows0 + nfull_o * 128, :].rearrange(
                                "(t p) d -> p t d", p=128),
                            in_=ost[:, :nfull_o * D].rearrange(
                                "p (t d) -> p t d", d=D))
                    if nb != 128:
                        nc.sync.dma_start(
                            out=out_d[rows0 + nfull_o * 128:rows0 + nfull_o * 128 + nb, :],
                            in_=ost[:nb, nfull_o * D:nfull_o * D + D])
                    ost = None

    nc.compile()
    return nc


# ---------------------------------------------------------------- host entry

def _host_prep(p: P, x, edge_index, Wn_w, Wn_b, Wg_w, Wg_b, Wa_w, Wa_b):
    x = np.asarray(x, np.float32)
    xT = np.ascontiguousarray(x.T).astype(BF16)
    tA, tB, LA, LB, LAg, LBg, per_core = prep_edges(
        p, np.asarray(edge_index)[0], np.asarray(edge_index)[1])

    shared = {
        "xT": xT,
        "WaW": np.asarray(Wa_w, np.float32).astype(BF16),
        "WaB": np.asarray(Wa_b, np.float32).reshape(p.D, 1),
        "WnW": np.asarray(Wn_w, np.float32).astype(BF16),
        "WgW": np.asarray(Wg_w, np.float32).astype(BF16),
        "biasNG": np.tile((np.asarray(Wn_b, np.float32)
                           + np.asarray(Wg_b, np.float32))[None, :], (p.D, 1)),
        "iota": np.tile(np.arange(p.D, dtype=np.float32)[None, :],
                        (p.D, 1)).astype(BF16),
        "ident": np.eye(p.D, dtype=np.float32).astype(BF16),
    }
    in_maps = []
    for k in range(p.NCORES):
        m = dict(shared)
        m["xT_own"] = np.ascontiguousarray(xT[:, k * p.NB:(k + 1) * p.NB])
        pc = per_core[k]
        m["idxA"], m["idxB"] = pc["idxA"], pc["idxB"]
        m["lrA"], m["lrB"] = pc["lrA"], pc["lrB"]
        in_maps.append(m)
    return tA, tB, LA, LB, LAg, LBg, in_maps


TRACE = False      # set True (e.g. from test.py) to capture an NTFF profile
LAST = None        # last BassKernelResults, for profiling/inspection


def kernel(**inputs) -> np.ndarray:
    global LAST
    from concourse import bass_utils
    bass_utils.upload_artifacts = lambda tmpdir: "local://" + tmpdir

    p = P()
    tA, tB, LA, LB, LAg, LBg, in_maps = _host_prep(p, **inputs)
    nc = build(p, tA, tB, LA, LB, LAg, LBg)
    kw = dict(trace=True, trace_cores=list(range(p.NCORES))) if TRACE else {}
    res = bass_utils.run_bass_kernel_spmd(
        nc, in_maps, core_ids=list(range(p.NCORES)), **kw)
    LAST = res
    out = np.concatenate([res.results[k]["out"] for k in range(p.NCORES)], axis=0)
    return out.astype(np.float32)


# revision 7
# speedup vs baseline: 1.7184x; 1.7184x over previous
"""AttentiveFPConv GNN message-passing kernel for 8 Trainium2 NeuronCores.

Reference computation (all fp32):
    alpha = sigmoid(x[col] @ Wa_w + Wa_b)          # per-edge attention
    neigh = x[col] * alpha                          # per-edge message
    aggr  = segment_sum(neigh, row, N)              # per-node aggregation
    out   = tanh(x @ Wn_w + Wn_b + aggr @ Wg_w + Wg_b)

Key algebraic identity: alpha depends only on the source node, so
    h = x * sigmoid(x @ Wa_w + Wa_b)                # per-NODE tensor
    aggr[n] = sum_{e: row[e]=n} h[col[e]]           # gather + segment-sum

Sharding: destination-node sharding. Core k owns nodes [5000k, 5000(k+1))
and ALL edges targeting them (balanced: rows are uniform). No collective
needed: each core computes its own aggr and output slice.

Per-core pipeline:
  Phase 1: h = x*sigmoid(x@Wa+b) for ALL nodes (replicated), h -> HBM bf16.
  Phase 2: dma_gather h[col] in destination-sorted edge order (4 SWDGE
           queues); segment-sum via one-hot matmuls accumulating aggr^T in
           PSUM per 128-node block. One-hot M built by DVE tensor_scalar
           reading a PSUM-resident iota (1-port mode: avoids the exclusive
           DVE<->GpSimd shared-SBUF-port lock that otherwise serializes
           against Q7 gather descriptor generation).
           (dma_gather indices are int16, so edges are split into two
           streams by col < 32768, each gathered against a rebased view.)
  Phase 3: out = tanh(x@Wn + aggr@Wg + ones x bias) -- bias added by a
           rank-1 matmul into the same PSUM accumulation group.
"""

import numpy as np
import ml_dtypes

BF16 = ml_dtypes.bfloat16

# ---------------------------------------------------------------- parameters

class P:
    """Problem/kernel parameters (full-size defaults; shrinkable for tests)."""
    def __init__(self, N=40000, D=128, NCORES=8, HSPLIT=32768,
                 GCHUNK=1024, PH1_CHUNK=2048, NQ=4):
        assert D == 128
        self.N, self.D, self.NCORES = N, D, NCORES
        self.NB = N // NCORES                 # nodes per core
        self.HSPLIT = HSPLIT                  # col split for int16 gather idx
        self.GCHUNK = GCHUNK                  # idxs per dma_gather (HW limit ~1024)
        self.GT = GCHUNK // 128               # gather tiles per chunk
        self.PH1_CHUNK = PH1_CHUNK            # nodes per phase-1 xT chunk
        self.NBLK = (self.NB + 127) // 128    # 128-node blocks per core
        self.NQ = NQ                          # SWDGE queues for dma_gather


# ------------------------------------------------------------ host edge prep

def prep_edges(p: P, row: np.ndarray, col: np.ndarray):
    """Per-core destination-sorted, block-padded edge streams."""
    row = np.asarray(row).astype(np.int64)
    col = np.asarray(col).astype(np.int64)
    cores = []
    for k in range(p.NCORES):
        sel = (row // p.NB) == k
        r = (row[sel] - k * p.NB).astype(np.int32)
        c = col[sel].astype(np.int32)
        order = np.argsort(r, kind="stable")
        r, c = r[order], c[order]
        lo = np.searchsorted(r, np.arange(p.NBLK) * 128)
        hi = np.searchsorted(r, np.minimum(np.arange(1, p.NBLK + 1) * 128, p.NB))
        blocks = []
        for b in range(p.NBLK):
            rb = r[lo[b]:hi[b]] - b * 128
            cb = c[lo[b]:hi[b]]
            mA = cb < p.HSPLIT
            blocks.append(((cb[mA], rb[mA]), (cb[~mA] - p.HSPLIT, rb[~mA])))
        cores.append(blocks)

    nA = np.array([[len(cores[k][b][0][0]) for b in range(p.NBLK)]
                   for k in range(p.NCORES)])
    nB = np.array([[len(cores[k][b][1][0]) for b in range(p.NBLK)]
                   for k in range(p.NCORES)])
    tA = np.maximum(1, -(-nA.max(axis=0) // 128))          # [NBLK]
    tB = np.maximum(1, -(-nB.max(axis=0) // 128))

    LA, LB = int(tA.sum()) * 128, int(tB.sum()) * 128
    LAg = -(-LA // p.GCHUNK) * p.GCHUNK
    LBg = -(-LB // p.GCHUNK) * p.GCHUNK

    per_core = []
    for k in range(p.NCORES):
        idxA = np.zeros(LAg, np.int16); lrA = np.full(LA, -1.0, np.float32)
        idxB = np.zeros(LBg, np.int16); lrB = np.full(LB, -1.0, np.float32)
        oA = oB = 0
        for b in range(p.NBLK):
            (cA, rA), (cB, rB) = cores[k][b]
            idxA[oA:oA + len(cA)] = cA; lrA[oA:oA + len(rA)] = rA
            oA += int(tA[b]) * 128
            idxB[oB:oB + len(cB)] = cB; lrB[oB:oB + len(rB)] = rB
            oB += int(tB[b]) * 128
        per_core.append({
            "idxA": np.tile(idxA.reshape(-1, 16).T, (8, 1)),   # [128, LAg/16]
            "idxB": np.tile(idxB.reshape(-1, 16).T, (8, 1)),
            "lrA": lrA.reshape(-1, 128).T.copy(),              # [128, LA/128]
            "lrB": lrB.reshape(-1, 128).T.copy(),
        })
    return tA, tB, LA, LB, LAg, LBg, per_core


# ------------------------------------------------------------- device kernel

def build(p: P, tA, tB, LA, LB, LAg, LBg):
    from concourse import bacc, mybir, tile

    f32, bf16, i16 = mybir.dt.float32, mybir.dt.bfloat16, mybir.dt.int16
    AF = mybir.ActivationFunctionType
    nc = bacc.Bacc("TRN2", target_bir_lowering=False, debug=False,
                   num_devices=p.NCORES, num_swdge_queues=p.NQ)

    N, D, NB, NBLK = p.N, p.D, p.NB, p.NBLK

    xT_d   = nc.dram_tensor("xT", [D, N], bf16, kind="ExternalInput")
    xTo_d  = nc.dram_tensor("xT_own", [D, NB], bf16, kind="ExternalInput")
    WaW_d  = nc.dram_tensor("WaW", [D, D], bf16, kind="ExternalInput")
    WaB_d  = nc.dram_tensor("WaB", [D, 1], f32, kind="ExternalInput")
    WnW_d  = nc.dram_tensor("WnW", [D, D], bf16, kind="ExternalInput")
    WgW_d  = nc.dram_tensor("WgW", [D, D], bf16, kind="ExternalInput")
    bias_d = nc.dram_tensor("biasR", [1, D], bf16, kind="ExternalInput")
    ones_d = nc.dram_tensor("onesR", [1, D], bf16, kind="ExternalInput")
    iota_d = nc.dram_tensor("iota", [D, D], f32, kind="ExternalInput")
    ident_d= nc.dram_tensor("ident", [D, D], bf16, kind="ExternalInput")
    idxA_d = nc.dram_tensor("idxA", [128, LAg // 16], i16, kind="ExternalInput")
    idxB_d = nc.dram_tensor("idxB", [128, LBg // 16], i16, kind="ExternalInput")
    lrA_d  = nc.dram_tensor("lrA", [128, LA // 128], f32, kind="ExternalInput")
    lrB_d  = nc.dram_tensor("lrB", [128, LB // 128], f32, kind="ExternalInput")
    out_d  = nc.dram_tensor("out", [NB, D], f32, kind="ExternalOutput")
    h_d    = nc.dram_tensor("h", [N, D], bf16, kind="Internal")

    with tile.TileContext(nc) as tc:
        with (
            tc.tile_pool(name="const", bufs=1) as cpool,
            tc.tile_pool(name="xchunk", bufs=3) as xpool,
            tc.tile_pool(name="hstage", bufs=2) as hspool,
            tc.tile_pool(name="pg", bufs=2, space="PSUM") as pg_pool,
            tc.tile_pool(name="pt", bufs=1, space="PSUM") as pt_pool,
            tc.tile_pool(name="pa", bufs=2, space="PSUM") as pa_pool,
            tc.tile_pool(name="po", bufs=2, space="PSUM") as po_pool,
            tc.tile_pool(name="piota", bufs=1, space="PSUM") as pi_pool,
            tc.tile_pool(name="sA", bufs=16) as gApool,
            tc.tile_pool(name="sB", bufs=8) as gBpool,
            tc.tile_pool(name="m", bufs=6) as mpool,
            tc.tile_pool(name="agg", bufs=3) as aggpool,
            tc.tile_pool(name="ph1w", bufs=3) as w1pool,
            tc.tile_pool(name="ostage", bufs=2) as ospool,
        ):
            # ---- constants into SBUF
            WaW = cpool.tile([D, D], bf16); nc.sync.dma_start(out=WaW[:], in_=WaW_d[:])
            WaB = cpool.tile([D, 1], f32); nc.sync.dma_start(out=WaB[:], in_=WaB_d[:])
            WnW = cpool.tile([D, D], bf16); nc.sync.dma_start(out=WnW[:], in_=WnW_d[:])
            WgW = cpool.tile([D, D], bf16); nc.sync.dma_start(out=WgW[:], in_=WgW_d[:])
            biasR = cpool.tile([1, D], bf16); nc.sync.dma_start(out=biasR[:], in_=bias_d[:])
            onesR = cpool.tile([1, D], bf16); nc.sync.dma_start(out=onesR[:], in_=ones_d[:])
            iota = cpool.tile([D, D], f32); nc.sync.dma_start(out=iota[:], in_=iota_d[:])
            ident = cpool.tile([D, D], bf16); nc.sync.dma_start(out=ident[:], in_=ident_d[:])
            xT_own = cpool.tile([D, NB], bf16); nc.sync.dma_start(out=xT_own[:], in_=xTo_d[:])
            idxA_sb = cpool.tile([128, LAg // 16], i16)
            nc.sync.dma_start(out=idxA_sb[:], in_=idxA_d[:])
            idxB_sb = cpool.tile([128, LBg // 16], i16)
            nc.sync.dma_start(out=idxB_sb[:], in_=idxB_d[:])
            lrA_sb = cpool.tile([128, LA // 128], f32)
            nc.sync.dma_start(out=lrA_sb[:], in_=lrA_d[:])
            lrB_sb = cpool.tile([128, LB // 128], f32)
            nc.sync.dma_start(out=lrB_sb[:], in_=lrB_d[:])
            # PSUM-resident iota (keeps M-build tensor_scalar off the shared
            # DVE<->GpSimd SBUF port pair during gather descriptor generation)
            iota_ps = pi_pool.tile([D, D], f32)
            nc.scalar.activation(iota_ps[:], iota[:], AF.Copy)

            # ---- phase 1: h = x * sigmoid(x@Wa + b), all N nodes, h -> HBM
            base = 0
            while base < N:
                cn = min(p.PH1_CHUNK, N - base)
                nfull = cn // 128
                rem = cn - nfull * 128
                xc = xpool.tile([D, p.PH1_CHUNK], bf16, tag="xc")
                nc.sync.dma_start(out=xc[:, :cn], in_=xT_d[:, base:base + cn])
                hst = hspool.tile([128, p.PH1_CHUNK], bf16, tag="hst")
                # groups of up to 4 full blocks share one PSUM bank
                g0 = 0
                while g0 < nfull:
                    gn = min(4, nfull - g0)
                    w = gn * 128
                    co = g0 * 128
                    pg = pg_pool.tile([D, 512], f32, tag="pg")
                    for q in range(gn):
                        nc.tensor.matmul(pg[:, q * 128:(q + 1) * 128], lhsT=WaW[:],
                                         rhs=xc[:, co + q * 128:co + (q + 1) * 128],
                                         start=True, stop=True)
                    sT = w1pool.tile([D, 512], bf16, tag="sT")
                    nc.scalar.activation(sT[:, :w], pg[:, :w], AF.Sigmoid,
                                         bias=WaB[:, 0:1])
                    hT = w1pool.tile([D, 512], bf16, tag="hT")
                    nc.vector.tensor_tensor(out=hT[:, :w], in0=xc[:, co:co + w],
                                            in1=sT[:, :w], op=mybir.AluOpType.mult)
                    pt = pt_pool.tile([128, 512], bf16, tag="pt")
                    for q in range(gn):
                        nc.tensor.transpose(pt[:, q * 128:(q + 1) * 128],
                                            hT[:, q * 128:q * 128 + 128], ident[:])
                    nc.vector.tensor_copy(out=hst[:, co:co + w], in_=pt[:, :w])
                    g0 += gn
                if rem:
                    co = nfull * 128
                    pg = pg_pool.tile([D, 512], f32, tag="pg")
                    nc.tensor.matmul(pg[:, :rem], lhsT=WaW[:],
                                     rhs=xc[:, co:co + rem], start=True, stop=True)
                    sT = w1pool.tile([D, 512], bf16, tag="sT")
                    nc.scalar.activation(sT[:, :rem], pg[:, :rem], AF.Sigmoid,
                                         bias=WaB[:, 0:1])
                    hT = w1pool.tile([D, 512], bf16, tag="hT")
                    nc.vector.tensor_tensor(out=hT[:, :rem], in0=xc[:, co:co + rem],
                                            in1=sT[:, :rem], op=mybir.AluOpType.mult)
                    pt = pt_pool.tile([128, 512], bf16, tag="pt")
                    nc.tensor.transpose(pt[:rem, :128], hT[:, :rem], ident[:])
                    nc.vector.tensor_copy(out=hst[:rem, co:co + 128],
                                          in_=pt[:rem, :128])
                if nfull:
                    nc.sync.dma_start(
                        out=h_d[base:base + nfull * 128, :].rearrange(
                            "(t p) d -> p t d", p=128),
                        in_=hst[:, :nfull * 128].rearrange(
                            "p (t d) -> p t d", d=128))
                if rem:
                    nc.sync.dma_start(
                        out=h_d[base + nfull * 128:base + cn, :],
                        in_=hst[:rem, nfull * 128:nfull * 128 + 128])
                base += cn

            # ---- phases 2+3, groups of 4 output blocks sharing PSUM banks
            nq_counter = [0]
            gA_tiles = [None] * (LAg // p.GCHUNK)
            gB_tiles = [None] * (LBg // p.GCHUNK)

            def ensure_chunk(tiles, which, ci):
                if tiles[ci] is not None:
                    return
                g = (gApool if which == "A" else gBpool).tile(
                    [128, p.GT, D], bf16, tag="g" + which)
                idx_sb = idxA_sb if which == "A" else idxB_sb
                src = h_d[:, :] if which == "A" else h_d[p.HSPLIT:, :]
                c0 = ci * (p.GCHUNK // 16)
                nc.gpsimd.dma_gather(
                    out_ap=g[:], in_ap=src, idxs_ap=idx_sb[:, c0:c0 + p.GCHUNK // 16],
                    num_idxs=p.GCHUNK, num_idxs_reg=p.GCHUNK, elem_size=D,
                    queue_num=nq_counter[0] % p.NQ)
                nq_counter[0] += 1
                tiles[ci] = g

            posA = np.concatenate([[0], np.cumsum(tA)]).astype(int)
            posB = np.concatenate([[0], np.cumsum(tB)]).astype(int)
            OCH = 8  # output blocks per staged DMA
            ost = None
            ost_base = 0
            ost_n = 0
            b0 = 0
            while b0 < NBLK:
                gn = min(4, NBLK - b0)
                pa = pa_pool.tile([D, 512], f32, tag="pa")
                po = po_pool.tile([128, 512], f32, tag="po")
                for q in range(gn):
                    b = b0 + q
                    nb = min(128, NB - b * 128)
                    ntiles = int(tA[b]) + int(tB[b])
                    ti = 0
                    for which, tcnt, pos, lr_sb, tiles in (
                            ("A", int(tA[b]), posA, lrA_sb, gA_tiles),
                            ("B", int(tB[b]), posB, lrB_sb, gB_tiles)):
                        for j in range(tcnt):
                            g = pos[b] + j
                            ensure_chunk(tiles, which, g // p.GT)
                            neigh = tiles[g // p.GT][:, g % p.GT, :]
                            M = mpool.tile([128, 128], bf16, tag="M")
                            nc.vector.tensor_scalar(
                                out=M[:], in0=iota_ps[:], scalar1=lr_sb[:, g:g + 1],
                                scalar2=None, op0=mybir.AluOpType.is_equal)
                            nc.tensor.matmul(pa[:, q * 128:(q + 1) * 128],
                                             lhsT=neigh, rhs=M[:],
                                             start=(ti == 0), stop=(ti == ntiles - 1))
                            ti += 1
                aggT = aggpool.tile([D, 512], bf16, tag="aggT")
                nc.vector.tensor_copy(out=aggT[:], in_=pa[:])
                # phase 3 for this group of blocks
                for q in range(gn):
                    b = b0 + q
                    nb = min(128, NB - b * 128)
                    sl = slice(q * 128, q * 128 + D)
                    nc.tensor.matmul(po[:nb, sl],
                                     lhsT=xT_own[:, b * 128:b * 128 + nb],
                                     rhs=WnW[:], start=True, stop=False)
                    nc.tensor.matmul(po[:nb, sl],
                                     lhsT=aggT[:, q * 128:q * 128 + nb],
                                     rhs=WgW[:], start=False, stop=False)
                    nc.tensor.matmul(po[:nb, sl], lhsT=onesR[:1, :nb],
                                     rhs=biasR[:1, :], start=False, stop=True)
                for q in range(gn):
                    b = b0 + q
                    nb = min(128, NB - b * 128)
                    if ost is None:
                        ost = ospool.tile([128, OCH * D], f32, tag="ost")
                        ost_base = b
                        ost_n = 0
                    nc.scalar.activation(ost[:nb, ost_n * D:(ost_n + 1) * D],
                                         po[:nb, q * 128:q * 128 + D], AF.Tanh)
                    ost_n += 1
                    if (ost_n == OCH) or (b == NBLK - 1):
                        rows0 = ost_base * 128
                        nfull_o = ost_n if nb == 128 else ost_n - 1
                        if nfull_o:
                            nc.sync.dma_start(
                                out=out_d[rows0:rows0 + nfull_o * 128, :].rearrange(
                                    "(t p) d -> p t d", p=128),
                                in_=ost[:, :nfull_o * D].rearrange(
                                    "p (t d) -> p t d", d=D))
                        if nb != 128:
                            nc.sync.dma_start(
                                out=out_d[rows0 + nfull_o * 128:
                                          rows0 + nfull_o * 128 + nb, :],
                                in_=ost[:nb, nfull_o * D:nfull_o * D + D])
                        ost = None
                b0 += gn

    nc.compile()
    return nc


# ---------------------------------------------------------------- host entry

def _host_prep(p: P, x, edge_index, Wn_w, Wn_b, Wg_w, Wg_b, Wa_w, Wa_b):
    x = np.asarray(x, np.float32)
    xT = np.ascontiguousarray(x.T).astype(BF16)
    tA, tB, LA, LB, LAg, LBg, per_core = prep_edges(
        p, np.asarray(edge_index)[0], np.asarray(edge_index)[1])

    shared = {
        "xT": xT,
        "WaW": np.asarray(Wa_w, np.float32).astype(BF16),
        "WaB": np.asarray(Wa_b, np.float32).reshape(p.D, 1),
        "WnW": np.asarray(Wn_w, np.float32).astype(BF16),
        "WgW": np.asarray(Wg_w, np.float32).astype(BF16),
        "biasR": (np.asarray(Wn_b, np.float32)
                  + np.asarray(Wg_b, np.float32)).reshape(1, p.D).astype(BF16),
        "onesR": np.ones((1, p.D), BF16),
        "iota": np.tile(np.arange(p.D, dtype=np.float32)[None, :], (p.D, 1)),
        "ident": np.eye(p.D, dtype=np.float32).astype(BF16),
    }
    in_maps = []
    for k in range(p.NCORES):
        m = dict(shared)
        m["xT_own"] = np.ascontiguousarray(xT[:, k * p.NB:(k + 1) * p.NB])
        pc = per_core[k]
        m["idxA"], m["idxB"] = pc["idxA"], pc["idxB"]
        m["lrA"], m["lrB"] = pc["lrA"], pc["lrB"]
        in_maps.append(m)
    return tA, tB, LA, LB, LAg, LBg, in_maps


TRACE = False      # set True (e.g. from test.py) to capture an NTFF profile
LAST = None        # last BassKernelResults, for profiling/inspection


def kernel(**inputs) -> np.ndarray:
    global LAST
    from concourse import bass_utils
    bass_utils.upload_artifacts = lambda tmpdir: "local://" + tmpdir

    p = P()
    tA, tB, LA, LB, LAg, LBg, in_maps = _host_prep(p, **inputs)
    nc = build(p, tA, tB, LA, LB, LAg, LBg)
    kw = dict(trace=True, trace_cores=list(range(p.NCORES))) if TRACE else {}
    res = bass_utils.run_bass_kernel_spmd(
        nc, in_maps, core_ids=list(range(p.NCORES)), **kw)
    LAST = res
    out = np.concatenate([res.results[k]["out"] for k in range(p.NCORES)], axis=0)
    return out.astype(np.float32)


# revision 9
# speedup vs baseline: 1.8628x; 1.0840x over previous
"""AttentiveFPConv GNN message-passing kernel for 8 Trainium2 NeuronCores.

Reference computation (all fp32):
    alpha = sigmoid(x[col] @ Wa_w + Wa_b)          # per-edge attention
    neigh = x[col] * alpha                          # per-edge message
    aggr  = segment_sum(neigh, row, N)              # per-node aggregation
    out   = tanh(x @ Wn_w + Wn_b + aggr @ Wg_w + Wg_b)

Key algebraic identity: alpha depends only on the source node, so
    h = x * sigmoid(x @ Wa_w + Wa_b)                # per-NODE tensor
    aggr[n] = sum_{e: row[e]=n} h[col[e]]           # gather + segment-sum

Sharding: destination-node sharding. Core k owns nodes [5000k, 5000(k+1))
and ALL edges targeting them (balanced: rows are uniform). No collective
needed: each core computes its own aggr and output slice.

Per-core pipeline:
  Phase 1: h = x*sigmoid(x@Wa+b) for ALL nodes (replicated), h -> HBM bf16.
  Phase 2: dma_gather h[col] in destination-sorted edge order (4 SWDGE
           queues); segment-sum via one-hot matmuls accumulating aggr^T in
           PSUM per 128-node block. One-hot M built by DVE tensor_scalar
           reading a PSUM-resident iota (1-port mode: avoids the exclusive
           DVE<->GpSimd shared-SBUF-port lock that otherwise serializes
           against Q7 gather descriptor generation).
           (dma_gather indices are int16, so edges are split into two
           streams by col < 32768, each gathered against a rebased view.)
  Phase 3: out = tanh(x@Wn + aggr@Wg + ones x bias) -- bias added by a
           rank-1 matmul into the same PSUM accumulation group.
"""

import numpy as np
import ml_dtypes

BF16 = ml_dtypes.bfloat16

# ---------------------------------------------------------------- parameters

class P:
    """Problem/kernel parameters (full-size defaults; shrinkable for tests)."""
    def __init__(self, N=40000, D=128, NCORES=8, HSPLIT=32768,
                 GCHUNK=1024, PH1_CHUNK=2048, NQ=4):
        assert D == 128
        self.N, self.D, self.NCORES = N, D, NCORES
        self.NB = N // NCORES                 # nodes per core
        self.HSPLIT = HSPLIT                  # col split for int16 gather idx
        self.GCHUNK = GCHUNK                  # idxs per dma_gather (HW limit ~1024)
        self.GT = GCHUNK // 128               # gather tiles per chunk
        self.PH1_CHUNK = PH1_CHUNK            # nodes per phase-1 xT chunk
        self.NBLK = (self.NB + 127) // 128    # 128-node blocks per core
        self.NQ = NQ                          # SWDGE queues for dma_gather


# ------------------------------------------------------------ host edge prep

def prep_edges(p: P, row: np.ndarray, col: np.ndarray):
    """Per-core destination-sorted, block-padded edge streams."""
    row = np.asarray(row).astype(np.int64)
    col = np.asarray(col).astype(np.int64)
    cores = []
    for k in range(p.NCORES):
        sel = (row // p.NB) == k
        r = (row[sel] - k * p.NB).astype(np.int32)
        c = col[sel].astype(np.int32)
        order = np.argsort(r, kind="stable")
        r, c = r[order], c[order]
        lo = np.searchsorted(r, np.arange(p.NBLK) * 128)
        hi = np.searchsorted(r, np.minimum(np.arange(1, p.NBLK + 1) * 128, p.NB))
        blocks = []
        for b in range(p.NBLK):
            rb = r[lo[b]:hi[b]] - b * 128
            cb = c[lo[b]:hi[b]]
            mA = cb < p.HSPLIT
            blocks.append(((cb[mA], rb[mA]), (cb[~mA] - p.HSPLIT, rb[~mA])))
        cores.append(blocks)

    nA = np.array([[len(cores[k][b][0][0]) for b in range(p.NBLK)]
                   for k in range(p.NCORES)])
    nB = np.array([[len(cores[k][b][1][0]) for b in range(p.NBLK)]
                   for k in range(p.NCORES)])
    tA = np.maximum(1, -(-nA.max(axis=0) // 128))          # [NBLK]
    tB = np.maximum(1, -(-nB.max(axis=0) // 128))

    LA, LB = int(tA.sum()) * 128, int(tB.sum()) * 128
    LAg = -(-LA // p.GCHUNK) * p.GCHUNK
    LBg = -(-LB // p.GCHUNK) * p.GCHUNK

    per_core = []
    for k in range(p.NCORES):
        idxA = np.zeros(LAg, np.int16); lrA = np.full(LA, -1.0, np.float32)
        idxB = np.zeros(LBg, np.int16); lrB = np.full(LB, -1.0, np.float32)
        oA = oB = 0
        for b in range(p.NBLK):
            (cA, rA), (cB, rB) = cores[k][b]
            idxA[oA:oA + len(cA)] = cA; lrA[oA:oA + len(rA)] = rA
            oA += int(tA[b]) * 128
            idxB[oB:oB + len(cB)] = cB; lrB[oB:oB + len(rB)] = rB
            oB += int(tB[b]) * 128
        per_core.append({
            "idxA": np.tile(idxA.reshape(-1, 16).T, (8, 1)),   # [128, LAg/16]
            "idxB": np.tile(idxB.reshape(-1, 16).T, (8, 1)),
            "lrA": lrA.reshape(-1, 128).T.copy(),              # [128, LA/128]
            "lrB": lrB.reshape(-1, 128).T.copy(),
        })
    return tA, tB, LA, LB, LAg, LBg, per_core


# ------------------------------------------------------------- device kernel

def build(p: P, tA, tB, LA, LB, LAg, LBg):
    from concourse import bacc, mybir, tile

    f32, bf16, i16 = mybir.dt.float32, mybir.dt.bfloat16, mybir.dt.int16
    AF = mybir.ActivationFunctionType
    nc = bacc.Bacc("TRN2", target_bir_lowering=False, debug=False,
                   num_devices=p.NCORES, num_swdge_queues=p.NQ)

    N, D, NB, NBLK = p.N, p.D, p.NB, p.NBLK

    xT_d   = nc.dram_tensor("xT", [D, N], bf16, kind="ExternalInput")
    xTo_d  = nc.dram_tensor("xT_own", [D, NB], bf16, kind="ExternalInput")
    WaW_d  = nc.dram_tensor("WaW", [D, D], bf16, kind="ExternalInput")
    WaB_d  = nc.dram_tensor("WaB", [D, 1], f32, kind="ExternalInput")
    WnW_d  = nc.dram_tensor("WnW", [D, D], bf16, kind="ExternalInput")
    WgW_d  = nc.dram_tensor("WgW", [D, D], bf16, kind="ExternalInput")
    bias_d = nc.dram_tensor("biasR", [1, D], bf16, kind="ExternalInput")
    ones_d = nc.dram_tensor("onesR", [1, D], bf16, kind="ExternalInput")
    iota_d = nc.dram_tensor("iota", [D, D], bf16, kind="ExternalInput")
    ident_d= nc.dram_tensor("ident", [D, D], bf16, kind="ExternalInput")
    idxA_d = nc.dram_tensor("idxA", [128, LAg // 16], i16, kind="ExternalInput")
    idxB_d = nc.dram_tensor("idxB", [128, LBg // 16], i16, kind="ExternalInput")
    lrA_d  = nc.dram_tensor("lrA", [128, LA // 128], f32, kind="ExternalInput")
    lrB_d  = nc.dram_tensor("lrB", [128, LB // 128], f32, kind="ExternalInput")
    nlrA_d = nc.dram_tensor("nlrA", [128, LA // 128], f32, kind="ExternalInput")
    nlrB_d = nc.dram_tensor("nlrB", [128, LB // 128], f32, kind="ExternalInput")
    out_d  = nc.dram_tensor("out", [NB, D], f32, kind="ExternalOutput")
    h_d    = nc.dram_tensor("h", [N, D], bf16, kind="Internal")

    with tile.TileContext(nc) as tc:
        with (
            tc.tile_pool(name="const", bufs=1) as cpool,
            tc.tile_pool(name="xchunk", bufs=3) as xpool,
            tc.tile_pool(name="hstage", bufs=2) as hspool,
            tc.tile_pool(name="pg", bufs=2, space="PSUM") as pg_pool,
            tc.tile_pool(name="pt", bufs=1, space="PSUM") as pt_pool,
            tc.tile_pool(name="pa", bufs=2, space="PSUM") as pa_pool,
            tc.tile_pool(name="po", bufs=2, space="PSUM") as po_pool,
            tc.tile_pool(name="piota", bufs=1, space="PSUM") as pi_pool,
            tc.tile_pool(name="sA", bufs=16) as gApool,
            tc.tile_pool(name="sB", bufs=8) as gBpool,
            tc.tile_pool(name="m", bufs=12) as mpool,
            tc.tile_pool(name="agg", bufs=3) as aggpool,
            tc.tile_pool(name="ph1w", bufs=3) as w1pool,
            tc.tile_pool(name="ostage", bufs=2) as ospool,
        ):
            # ---- constants into SBUF
            WaW = cpool.tile([D, D], bf16); nc.sync.dma_start(out=WaW[:], in_=WaW_d[:])
            WaB = cpool.tile([D, 1], f32); nc.sync.dma_start(out=WaB[:], in_=WaB_d[:])
            WnW = cpool.tile([D, D], bf16); nc.sync.dma_start(out=WnW[:], in_=WnW_d[:])
            WgW = cpool.tile([D, D], bf16); nc.sync.dma_start(out=WgW[:], in_=WgW_d[:])
            biasR = cpool.tile([1, D], bf16); nc.sync.dma_start(out=biasR[:], in_=bias_d[:])
            onesR = cpool.tile([1, D], bf16); nc.sync.dma_start(out=onesR[:], in_=ones_d[:])
            iota = cpool.tile([D, D], bf16); nc.sync.dma_start(out=iota[:], in_=iota_d[:])
            ident = cpool.tile([D, D], bf16); nc.sync.dma_start(out=ident[:], in_=ident_d[:])
            xT_own = cpool.tile([D, NB], bf16); nc.sync.dma_start(out=xT_own[:], in_=xTo_d[:])
            idxA_sb = cpool.tile([128, LAg // 16], i16)
            nc.sync.dma_start(out=idxA_sb[:], in_=idxA_d[:])
            idxB_sb = cpool.tile([128, LBg // 16], i16)
            nc.sync.dma_start(out=idxB_sb[:], in_=idxB_d[:])
            lrA_sb = cpool.tile([128, LA // 128], f32)
            nc.sync.dma_start(out=lrA_sb[:], in_=lrA_d[:])
            lrB_sb = cpool.tile([128, LB // 128], f32)
            nc.sync.dma_start(out=lrB_sb[:], in_=lrB_d[:])
            nlrA_sb = cpool.tile([128, LA // 128], f32)
            nc.sync.dma_start(out=nlrA_sb[:], in_=nlrA_d[:])
            nlrB_sb = cpool.tile([128, LB // 128], f32)
            nc.sync.dma_start(out=nlrB_sb[:], in_=nlrB_d[:])
            # PSUM-resident iota (keeps M-build tensor_scalar off the shared
            # DVE<->GpSimd SBUF port pair during gather descriptor generation)
            iota_ps = pi_pool.tile([D, D], bf16)
            nc.tensor.transpose(iota_ps[:], iota[:], ident[:])

            # ---- phase 1: h = x * sigmoid(x@Wa + b), all N nodes, h -> HBM
            base = 0
            while base < N:
                cn = min(p.PH1_CHUNK, N - base)
                nfull = cn // 128
                rem = cn - nfull * 128
                xc = xpool.tile([D, p.PH1_CHUNK], bf16, tag="xc")
                nc.sync.dma_start(out=xc[:, :cn], in_=xT_d[:, base:base + cn])
                hst = hspool.tile([128, p.PH1_CHUNK], bf16, tag="hst")
                # groups of up to 4 full blocks share one PSUM bank
                g0 = 0
                while g0 < nfull:
                    gn = min(4, nfull - g0)
                    w = gn * 128
                    co = g0 * 128
                    pg = pg_pool.tile([D, 512], f32, tag="pg")
                    nc.tensor.matmul(pg[:, :w], lhsT=WaW[:],
                                     rhs=xc[:, co:co + w], start=True, stop=True)
                    sT = w1pool.tile([D, 512], bf16, tag="sT")
                    nc.scalar.activation(sT[:, :w], pg[:, :w], AF.Sigmoid,
                                         bias=WaB[:, 0:1])
                    hT = w1pool.tile([D, 512], bf16, tag="hT")
                    nc.vector.tensor_tensor(out=hT[:, :w], in0=xc[:, co:co + w],
                                            in1=sT[:, :w], op=mybir.AluOpType.mult)
                    pt = pt_pool.tile([128, 512], bf16, tag="pt")
                    for q in range(gn):
                        nc.tensor.transpose(pt[:, q * 128:(q + 1) * 128],
                                            hT[:, q * 128:q * 128 + 128], ident[:])
                    nc.vector.tensor_copy(out=hst[:, co:co + w], in_=pt[:, :w])
                    g0 += gn
                if rem:
                    co = nfull * 128
                    pg = pg_pool.tile([D, 512], f32, tag="pg")
                    nc.tensor.matmul(pg[:, :rem], lhsT=WaW[:],
                                     rhs=xc[:, co:co + rem], start=True, stop=True)
                    sT = w1pool.tile([D, 512], bf16, tag="sT")
                    nc.scalar.activation(sT[:, :rem], pg[:, :rem], AF.Sigmoid,
                                         bias=WaB[:, 0:1])
                    hT = w1pool.tile([D, 512], bf16, tag="hT")
                    nc.vector.tensor_tensor(out=hT[:, :rem], in0=xc[:, co:co + rem],
                                            in1=sT[:, :rem], op=mybir.AluOpType.mult)
                    pt = pt_pool.tile([128, 512], bf16, tag="pt")
                    nc.tensor.transpose(pt[:rem, :128], hT[:, :rem], ident[:])
                    nc.vector.tensor_copy(out=hst[:rem, co:co + 128],
                                          in_=pt[:rem, :128])
                if nfull:
                    nc.sync.dma_start(
                        out=h_d[base:base + nfull * 128, :].rearrange(
                            "(t p) d -> p t d", p=128),
                        in_=hst[:, :nfull * 128].rearrange(
                            "p (t d) -> p t d", d=128))
                if rem:
                    nc.sync.dma_start(
                        out=h_d[base + nfull * 128:base + cn, :],
                        in_=hst[:rem, nfull * 128:nfull * 128 + 128])
                base += cn

            # ---- phases 2+3, groups of 4 output blocks sharing PSUM banks
            nq_counter = [0]
            gA_tiles = [None] * (LAg // p.GCHUNK)
            gB_tiles = [None] * (LBg // p.GCHUNK)

            def ensure_chunk(tiles, which, ci):
                if tiles[ci] is not None:
                    return
                g = (gApool if which == "A" else gBpool).tile(
                    [128, p.GT, D], bf16, tag="g" + which)
                idx_sb = idxA_sb if which == "A" else idxB_sb
                src = h_d[:, :] if which == "A" else h_d[p.HSPLIT:, :]
                c0 = ci * (p.GCHUNK // 16)
                nc.gpsimd.dma_gather(
                    out_ap=g[:], in_ap=src, idxs_ap=idx_sb[:, c0:c0 + p.GCHUNK // 16],
                    num_idxs=p.GCHUNK, num_idxs_reg=p.GCHUNK, elem_size=D,
                    queue_num=nq_counter[0] % p.NQ)
                nq_counter[0] += 1
                tiles[ci] = g

            posA = np.concatenate([[0], np.cumsum(tA)]).astype(int)
            posB = np.concatenate([[0], np.cumsum(tB)]).astype(int)
            OCH = 8  # output blocks per staged DMA
            ost = None
            ost_base = 0
            ost_n = 0
            b0 = 0
            while b0 < NBLK:
                gn = min(4, NBLK - b0)
                pa = pa_pool.tile([D, 512], f32, tag="pa")
                po = po_pool.tile([128, 512], f32, tag="po")
                for q in range(gn):
                    b = b0 + q
                    nb = min(128, NB - b * 128)
                    ntiles = int(tA[b]) + int(tB[b])
                    ti = 0
                    for which, tcnt, pos, lr_sb, nlr_sb, tiles in (
                            ("A", int(tA[b]), posA, lrA_sb, nlrA_sb, gA_tiles),
                            ("B", int(tB[b]), posB, lrB_sb, nlrB_sb, gB_tiles)):
                        for j in range(tcnt):
                            g = pos[b] + j
                            ensure_chunk(tiles, which, g // p.GT)
                            neigh = tiles[g // p.GT][:, g % p.GT, :]
                            M = mpool.tile([128, 128], bf16, tag="M")
                            if ti % 4 == 3:
                                # ACT path: one-hot = relu(1 - (iota - lr)^2)
                                sq = mpool.tile([128, 128], f32, tag="sq")
                                nc.scalar.activation(sq[:], iota_ps[:], AF.Square,
                                                     bias=nlr_sb[:, g:g + 1])
                                nc.scalar.activation(M[:], sq[:], AF.Relu,
                                                     scale=-1.0, bias=1.0)
                            else:
                                nc.vector.tensor_scalar(
                                    out=M[:], in0=iota_ps[:],
                                    scalar1=lr_sb[:, g:g + 1],
                                    scalar2=None, op0=mybir.AluOpType.is_equal)
                            nc.tensor.matmul(pa[:, q * 128:(q + 1) * 128],
                                             lhsT=neigh, rhs=M[:],
                                             start=(ti == 0), stop=(ti == ntiles - 1))
                            ti += 1
                aggT = aggpool.tile([D, 512], bf16, tag="aggT")
                nc.vector.tensor_copy(out=aggT[:], in_=pa[:])
                # phase 3 for this group of blocks
                for q in range(gn):
                    b = b0 + q
                    nb = min(128, NB - b * 128)
                    sl = slice(q * 128, q * 128 + D)
                    nc.tensor.matmul(po[:nb, sl],
                                     lhsT=xT_own[:, b * 128:b * 128 + nb],
                                     rhs=WnW[:], start=True, stop=False)
                    nc.tensor.matmul(po[:nb, sl],
                                     lhsT=aggT[:, q * 128:q * 128 + nb],
                                     rhs=WgW[:], start=False, stop=False)
                    nc.tensor.matmul(po[:nb, sl], lhsT=onesR[:1, :nb],
                                     rhs=biasR[:1, :], start=False, stop=True)
                for q in range(gn):
                    b = b0 + q
                    nb = min(128, NB - b * 128)
                    if ost is None:
                        ost = ospool.tile([128, OCH * D], f32, tag="ost")
                        ost_base = b
                        ost_n = 0
                    nc.scalar.activation(ost[:nb, ost_n * D:(ost_n + 1) * D],
                                         po[:nb, q * 128:q * 128 + D], AF.Tanh)
                    ost_n += 1
                    if (ost_n == OCH) or (b == NBLK - 1):
                        rows0 = ost_base * 128
                        nfull_o = ost_n if nb == 128 else ost_n - 1
                        if nfull_o:
                            nc.sync.dma_start(
                                out=out_d[rows0:rows0 + nfull_o * 128, :].rearrange(
                                    "(t p) d -> p t d", p=128),
                                in_=ost[:, :nfull_o * D].rearrange(
                                    "p (t d) -> p t d", d=D))
                        if nb != 128:
                            nc.sync.dma_start(
                                out=out_d[rows0 + nfull_o * 128:
                                          rows0 + nfull_o * 128 + nb, :],
                                in_=ost[:nb, nfull_o * D:nfull_o * D + D])
                        ost = None
                b0 += gn

    nc.compile()
    return nc


# ---------------------------------------------------------------- host entry

def _host_prep(p: P, x, edge_index, Wn_w, Wn_b, Wg_w, Wg_b, Wa_w, Wa_b):
    x = np.asarray(x, np.float32)
    xT = np.ascontiguousarray(x.T).astype(BF16)
    tA, tB, LA, LB, LAg, LBg, per_core = prep_edges(
        p, np.asarray(edge_index)[0], np.asarray(edge_index)[1])

    shared = {
        "xT": xT,
        "WaW": np.asarray(Wa_w, np.float32).astype(BF16),
        "WaB": np.asarray(Wa_b, np.float32).reshape(p.D, 1),
        "WnW": np.asarray(Wn_w, np.float32).astype(BF16),
        "WgW": np.asarray(Wg_w, np.float32).astype(BF16),
        "biasR": (np.asarray(Wn_b, np.float32)
                  + np.asarray(Wg_b, np.float32)).reshape(1, p.D).astype(BF16),
        "onesR": np.ones((1, p.D), BF16),
        "iota": np.tile(np.arange(p.D, dtype=np.float32)[:, None],
                        (1, p.D)).astype(BF16),
        "ident": np.eye(p.D, dtype=np.float32).astype(BF16),
    }
    in_maps = []
    for k in range(p.NCORES):
        m = dict(shared)
        m["xT_own"] = np.ascontiguousarray(xT[:, k * p.NB:(k + 1) * p.NB])
        pc = per_core[k]
        m["idxA"], m["idxB"] = pc["idxA"], pc["idxB"]
        m["lrA"], m["lrB"] = pc["lrA"], pc["lrB"]
        m["nlrA"], m["nlrB"] = -pc["lrA"], -pc["lrB"]
        in_maps.append(m)
    return tA, tB, LA, LB, LAg, LBg, in_maps


TRACE = False      # set True (e.g. from test.py) to capture an NTFF profile
LAST = None        # last BassKernelResults, for profiling/inspection


def kernel(**inputs) -> np.ndarray:
    global LAST
    from concourse import bass_utils
    bass_utils.upload_artifacts = lambda tmpdir: "local://" + tmpdir

    p = P()
    tA, tB, LA, LB, LAg, LBg, in_maps = _host_prep(p, **inputs)
    nc = build(p, tA, tB, LA, LB, LAg, LBg)
    kw = dict(trace=True, trace_cores=list(range(p.NCORES))) if TRACE else {}
    res = bass_utils.run_bass_kernel_spmd(
        nc, in_maps, core_ids=list(range(p.NCORES)), **kw)
    LAST = res
    out = np.concatenate([res.results[k]["out"] for k in range(p.NCORES)], axis=0)
    return out.astype(np.float32)


# revision 10
# speedup vs baseline: 2.0834x; 1.1185x over previous
"""AttentiveFPConv GNN message-passing kernel for 8 Trainium2 NeuronCores.

Reference computation (all fp32):
    alpha = sigmoid(x[col] @ Wa_w + Wa_b)          # per-edge attention
    neigh = x[col] * alpha                          # per-edge message
    aggr  = segment_sum(neigh, row, N)              # per-node aggregation
    out   = tanh(x @ Wn_w + Wn_b + aggr @ Wg_w + Wg_b)

Key algebraic identity: alpha depends only on the source node, so
    h = x * sigmoid(x @ Wa_w + Wa_b)                # per-NODE tensor
    aggr[n] = sum_{e: row[e]=n} h[col[e]]           # gather + segment-sum

Sharding: destination-node sharding. Core k owns nodes [5000k, 5000(k+1))
and ALL edges targeting them (balanced: rows are uniform). No collective
needed: each core computes its own aggr and output slice.

Per-core pipeline:
  Phase 1: h = x*sigmoid(x@Wa+b) for ALL nodes (replicated), h -> HBM bf16.
  Phase 2: dma_gather h[col] in destination-sorted edge order (4 SWDGE
           queues); segment-sum via one-hot matmuls accumulating aggr^T in
           PSUM per 128-node block. One-hot M built by DVE tensor_scalar
           reading a PSUM-resident iota (1-port mode: avoids the exclusive
           DVE<->GpSimd shared-SBUF-port lock that otherwise serializes
           against Q7 gather descriptor generation).
           (dma_gather indices are int16, so edges are split into two
           streams by col < 32768, each gathered against a rebased view.)
  Phase 3: out = tanh(x@Wn + aggr@Wg + ones x bias) -- bias added by a
           rank-1 matmul into the same PSUM accumulation group.
"""

import numpy as np
import ml_dtypes

BF16 = ml_dtypes.bfloat16

# ---------------------------------------------------------------- parameters

class P:
    """Problem/kernel parameters (full-size defaults; shrinkable for tests)."""
    def __init__(self, N=40000, D=128, NCORES=8, HSPLIT=19968,
                 GCHUNK=1024, PH1_CHUNK=2048, NQ=4):
        assert D == 128
        self.N, self.D, self.NCORES = N, D, NCORES
        self.NB = N // NCORES                 # nodes per core
        self.HSPLIT = HSPLIT                  # col split for int16 gather idx
        self.GCHUNK = GCHUNK                  # idxs per dma_gather (HW limit ~1024)
        self.GT = GCHUNK // 128               # gather tiles per chunk
        self.PH1_CHUNK = PH1_CHUNK            # nodes per phase-1 xT chunk
        self.NBLK = (self.NB + 127) // 128    # 128-node blocks per core
        self.NQ = NQ                          # SWDGE queues for dma_gather


# ------------------------------------------------------------ host edge prep

def prep_edges(p: P, row: np.ndarray, col: np.ndarray):
    """Per-core destination-sorted, block-padded edge streams."""
    row = np.asarray(row).astype(np.int64)
    col = np.asarray(col).astype(np.int64)
    cores = []
    for k in range(p.NCORES):
        sel = (row // p.NB) == k
        r = (row[sel] - k * p.NB).astype(np.int32)
        c = col[sel].astype(np.int32)
        order = np.argsort(r, kind="stable")
        r, c = r[order], c[order]
        lo = np.searchsorted(r, np.arange(p.NBLK) * 128)
        hi = np.searchsorted(r, np.minimum(np.arange(1, p.NBLK + 1) * 128, p.NB))
        blocks = []
        for b in range(p.NBLK):
            rb = r[lo[b]:hi[b]] - b * 128
            cb = c[lo[b]:hi[b]]
            mA = cb < p.HSPLIT
            blocks.append(((cb[mA], rb[mA]), (cb[~mA] - p.HSPLIT, rb[~mA])))
        cores.append(blocks)

    nA = np.array([[len(cores[k][b][0][0]) for b in range(p.NBLK)]
                   for k in range(p.NCORES)])
    nB = np.array([[len(cores[k][b][1][0]) for b in range(p.NBLK)]
                   for k in range(p.NCORES)])
    tA = np.maximum(1, -(-nA.max(axis=0) // 128))          # [NBLK]
    tB = np.maximum(1, -(-nB.max(axis=0) // 128))

    LA, LB = int(tA.sum()) * 128, int(tB.sum()) * 128
    LAg = -(-LA // p.GCHUNK) * p.GCHUNK
    LBg = -(-LB // p.GCHUNK) * p.GCHUNK

    per_core = []
    for k in range(p.NCORES):
        idxA = np.zeros(LAg, np.int16); lrA = np.full(LA, -1.0, np.float32)
        idxB = np.zeros(LBg, np.int16); lrB = np.full(LB, -1.0, np.float32)
        oA = oB = 0
        for b in range(p.NBLK):
            (cA, rA), (cB, rB) = cores[k][b]
            idxA[oA:oA + len(cA)] = cA; lrA[oA:oA + len(rA)] = rA
            oA += int(tA[b]) * 128
            idxB[oB:oB + len(cB)] = cB; lrB[oB:oB + len(rB)] = rB
            oB += int(tB[b]) * 128
        per_core.append({
            "idxA": np.tile(idxA.reshape(-1, 16).T, (8, 1)),   # [128, LAg/16]
            "idxB": np.tile(idxB.reshape(-1, 16).T, (8, 1)),
            "lrA": lrA.reshape(-1, 128).T.copy(),              # [128, LA/128]
            "lrB": lrB.reshape(-1, 128).T.copy(),
        })
    return tA, tB, LA, LB, LAg, LBg, per_core


# ------------------------------------------------------------- device kernel

def build(p: P, tA, tB, LA, LB, LAg, LBg):
    from concourse import bacc, mybir, tile

    f32, bf16, i16 = mybir.dt.float32, mybir.dt.bfloat16, mybir.dt.int16
    AF = mybir.ActivationFunctionType
    nc = bacc.Bacc("TRN2", target_bir_lowering=False, debug=False,
                   num_devices=p.NCORES, num_swdge_queues=p.NQ)

    N, D, NB, NBLK = p.N, p.D, p.NB, p.NBLK
    H = p.HSPLIT                    # h1 rows; h2 rows = N - H
    N2 = N - H
    assert H % 128 == 0

    xT_d   = nc.dram_tensor("xT", [D, N], bf16, kind="ExternalInput")
    xTo_d  = nc.dram_tensor("xT_own", [D, NB], bf16, kind="ExternalInput")
    WaW_d  = nc.dram_tensor("WaW", [D, D], bf16, kind="ExternalInput")
    WaB_d  = nc.dram_tensor("WaB", [D, 1], f32, kind="ExternalInput")
    WnW_d  = nc.dram_tensor("WnW", [D, D], bf16, kind="ExternalInput")
    WgW_d  = nc.dram_tensor("WgW", [D, D], bf16, kind="ExternalInput")
    bias_d = nc.dram_tensor("biasR", [1, D], bf16, kind="ExternalInput")
    ones_d = nc.dram_tensor("onesR", [1, D], bf16, kind="ExternalInput")
    iota_d = nc.dram_tensor("iota", [D, D], bf16, kind="ExternalInput")
    ident_d= nc.dram_tensor("ident", [D, D], bf16, kind="ExternalInput")
    idxA_d = nc.dram_tensor("idxA", [128, LAg // 16], i16, kind="ExternalInput")
    idxB_d = nc.dram_tensor("idxB", [128, LBg // 16], i16, kind="ExternalInput")
    lrA_d  = nc.dram_tensor("lrA", [128, LA // 128], f32, kind="ExternalInput")
    lrB_d  = nc.dram_tensor("lrB", [128, LB // 128], f32, kind="ExternalInput")
    nlrA_d = nc.dram_tensor("nlrA", [128, LA // 128], f32, kind="ExternalInput")
    nlrB_d = nc.dram_tensor("nlrB", [128, LB // 128], f32, kind="ExternalInput")
    out_d  = nc.dram_tensor("out", [NB, D], f32, kind="ExternalOutput")
    h1_d   = nc.dram_tensor("h1", [H, D], bf16, kind="Internal")
    h2_d   = nc.dram_tensor("h2", [N2, D], bf16, kind="Internal")

    PIECE = 9984                   # nodes per hT staging piece (78 blocks)

    with tile.TileContext(nc) as tc:
        with (
            tc.tile_pool(name="const", bufs=1) as cpool,
            tc.tile_pool(name="xchunk", bufs=3) as xpool,
            tc.tile_pool(name="hT", bufs=2) as htpool,
            tc.tile_pool(name="hstage", bufs=1) as hspool,
            tc.tile_pool(name="pg", bufs=2, space="PSUM") as pg_pool,
            tc.tile_pool(name="pt", bufs=1, space="PSUM") as pt_pool,
            tc.tile_pool(name="pa", bufs=2, space="PSUM") as pa_pool,
            tc.tile_pool(name="po", bufs=2, space="PSUM") as po_pool,
            tc.tile_pool(name="piota", bufs=1, space="PSUM") as pi_pool,
            tc.tile_pool(name="sA", bufs=12) as gApool,
            tc.tile_pool(name="sB", bufs=12) as gBpool,
            tc.tile_pool(name="m", bufs=16) as mpool,
            tc.tile_pool(name="aggA", bufs=(NBLK + 3) // 4) as aggApool,
            tc.tile_pool(name="aggB", bufs=3) as aggBpool,
            tc.tile_pool(name="ph1w", bufs=3) as w1pool,
            tc.tile_pool(name="ostage", bufs=2) as ospool,
        ):
            # ---- constants into SBUF
            WaW = cpool.tile([D, D], bf16); nc.sync.dma_start(out=WaW[:], in_=WaW_d[:])
            WaB = cpool.tile([D, 1], f32); nc.sync.dma_start(out=WaB[:], in_=WaB_d[:])
            WnW = cpool.tile([D, D], bf16); nc.sync.dma_start(out=WnW[:], in_=WnW_d[:])
            WgW = cpool.tile([D, D], bf16); nc.sync.dma_start(out=WgW[:], in_=WgW_d[:])
            biasR = cpool.tile([1, D], bf16); nc.sync.dma_start(out=biasR[:], in_=bias_d[:])
            onesR = cpool.tile([1, D], bf16); nc.sync.dma_start(out=onesR[:], in_=ones_d[:])
            iota = cpool.tile([D, D], bf16); nc.sync.dma_start(out=iota[:], in_=iota_d[:])
            ident = cpool.tile([D, D], bf16); nc.sync.dma_start(out=ident[:], in_=ident_d[:])
            xT_own = cpool.tile([D, NB], bf16); nc.sync.dma_start(out=xT_own[:], in_=xTo_d[:])
            idxA_sb = cpool.tile([128, LAg // 16], i16)
            nc.sync.dma_start(out=idxA_sb[:], in_=idxA_d[:])
            idxB_sb = cpool.tile([128, LBg // 16], i16)
            nc.sync.dma_start(out=idxB_sb[:], in_=idxB_d[:])
            lrA_sb = cpool.tile([128, LA // 128], f32)
            nc.sync.dma_start(out=lrA_sb[:], in_=lrA_d[:])
            lrB_sb = cpool.tile([128, LB // 128], f32)
            nc.sync.dma_start(out=lrB_sb[:], in_=lrB_d[:])
            nlrA_sb = cpool.tile([128, LA // 128], f32)
            nc.sync.dma_start(out=nlrA_sb[:], in_=nlrA_d[:])
            nlrB_sb = cpool.tile([128, LB // 128], f32)
            nc.sync.dma_start(out=nlrB_sb[:], in_=nlrB_d[:])
            iota_ps = pi_pool.tile([D, D], bf16)
            nc.tensor.transpose(iota_ps[:], iota[:], ident[:])

            # ---- phase 1: h = x * sigmoid(x@Wa + b); hT pieces -> xbar -> HBM
            def ph1_compute(hTp, base, cn):
                """Compute hT for nodes [base, base+cn) into hTp[:, :cn]."""
                off = 0
                while off < cn:
                    w = min(2048, cn - off)
                    xc = xpool.tile([D, 2048], bf16, tag="xc")
                    nc.sync.dma_start(out=xc[:, :w], in_=xT_d[:, base + off:base + off + w])
                    g0 = 0
                    while g0 < w:
                        gw = min(512, w - g0)
                        pg = pg_pool.tile([D, 512], f32, tag="pg")
                        nc.tensor.matmul(pg[:, :gw], lhsT=WaW[:],
                                         rhs=xc[:, g0:g0 + gw], start=True, stop=True)
                        sT = w1pool.tile([D, 512], bf16, tag="sT")
                        nc.scalar.activation(sT[:, :gw], pg[:, :gw], AF.Sigmoid,
                                             bias=WaB[:, 0:1])
                        nc.vector.tensor_tensor(out=hTp[:, off + g0:off + g0 + gw],
                                                in0=xc[:, g0:g0 + gw],
                                                in1=sT[:, :gw], op=mybir.AluOpType.mult)
                        g0 += gw
                    off += w

            def ph1_flush(hTp, h_t, base_in_h, cn):
                """xbar-transpose hTp[:, :cn] and DMA to h_t rows [base_in_h, +cn)."""
                nfull = cn // 128
                rem = cn - nfull * 128
                if nfull:
                    hst = hspool.tile([128, PIECE // 128, 128], bf16, tag="hst")
                    nc.sync.dma_start_transpose(hst[:, :nfull, :], hTp[:, :nfull * 128])
                    nc.sync.dma_start(
                        out=h_t[base_in_h:base_in_h + nfull * 128, :].rearrange(
                            "(t p) d -> p t d", p=128),
                        in_=hst[:, :nfull, :])
                if rem:
                    pt = pt_pool.tile([128, 128], bf16, tag="pt")
                    nc.tensor.transpose(pt[:rem, :], hTp[:, nfull * 128:nfull * 128 + rem],
                                        ident[:])
                    tl = w1pool.tile([128, 128], bf16, tag="tail")
                    nc.vector.tensor_copy(out=tl[:rem, :], in_=pt[:rem, :])
                    nc.sync.dma_start(
                        out=h_t[base_in_h + nfull * 128:base_in_h + cn, :],
                        in_=tl[:rem, :])

            # h1: nodes [0, H)
            base = 0
            while base < H:
                cn = min(PIECE, H - base)
                hTp = htpool.tile([D, PIECE], bf16, tag="hT")
                ph1_compute(hTp, base, cn)
                ph1_flush(hTp, h1_d, base, cn)
                base += cn
            # h2: nodes [H, N)
            while base < N:
                cn = min(PIECE, N - base)
                hTp = htpool.tile([D, PIECE], bf16, tag="hT")
                ph1_compute(hTp, base, cn)
                ph1_flush(hTp, h2_d, base - H, cn)
                base += cn

            # ---- phase 2: two passes (A from h1, B from h2), one-hot scatter
            nq_counter = [0]
            gA_tiles = [None] * (LAg // p.GCHUNK)
            gB_tiles = [None] * (LBg // p.GCHUNK)

            def ensure_chunk(tiles, which, ci):
                if tiles[ci] is not None:
                    return
                g = (gApool if which == "A" else gBpool).tile(
                    [128, p.GT, D], bf16, tag="g" + which)
                idx_sb = idxA_sb if which == "A" else idxB_sb
                src = h1_d[:, :] if which == "A" else h2_d[:, :]
                c0 = ci * (p.GCHUNK // 16)
                nc.gpsimd.dma_gather(
                    out_ap=g[:], in_ap=src, idxs_ap=idx_sb[:, c0:c0 + p.GCHUNK // 16],
                    num_idxs=p.GCHUNK, num_idxs_reg=p.GCHUNK, elem_size=D,
                    queue_num=nq_counter[0] % p.NQ)
                nq_counter[0] += 1
                tiles[ci] = g

            posA = np.concatenate([[0], np.cumsum(tA)]).astype(int)
            posB = np.concatenate([[0], np.cumsum(tB)]).astype(int)
            NG = (NBLK + 3) // 4

            def scatter_pass(tcnts, pos, lr_sb, nlr_sb, tiles, which, aggpool):
                """One-hot matmul scatter for one stream; returns agg tiles."""
                aggs = []
                b0 = 0
                while b0 < NBLK:
                    gn = min(4, NBLK - b0)
                    pa = pa_pool.tile([D, 512], f32, tag="pa")
                    for q in range(gn):
                        b = b0 + q
                        tcnt = int(tcnts[b])
                        for j in range(tcnt):
                            g = pos[b] + j
                            ensure_chunk(tiles, which, g // p.GT)
                            neigh = tiles[g // p.GT][:, g % p.GT, :]
                            M = mpool.tile([128, 128], bf16, tag="M")
                            if j % 4 == 3:
                                sq = mpool.tile([128, 128], f32, tag="sq")
                                nc.scalar.activation(sq[:], iota_ps[:], AF.Square,
                                                     bias=nlr_sb[:, g:g + 1])
                                nc.scalar.activation(M[:], sq[:], AF.Relu,
                                                     scale=-1.0, bias=1.0)
                            else:
                                nc.vector.tensor_scalar(
                                    out=M[:], in0=iota_ps[:],
                                    scalar1=lr_sb[:, g:g + 1],
                                    scalar2=None, op0=mybir.AluOpType.is_equal)
                            nc.tensor.matmul(pa[:, q * 128:(q + 1) * 128],
                                             lhsT=neigh, rhs=M[:],
                                             start=(j == 0), stop=(j == tcnt - 1))
                    agg = aggpool.tile([D, 512], bf16, tag="agg" + which)
                    nc.vector.tensor_copy(out=agg[:], in_=pa[:])
                    aggs.append(agg)
                    b0 += gn
                return aggs

            aggsA = scatter_pass(tA, posA, lrA_sb, nlrA_sb, gA_tiles, "A", aggApool)
            aggsB = []

            # ---- pass B + phase 3 fused per 4-block group
            OCH = 8
            ost = None
            ost_base = 0
            ost_n = 0
            b0 = 0
            while b0 < NBLK:
                gn = min(4, NBLK - b0)
                gi = b0 // 4
                pa = pa_pool.tile([D, 512], f32, tag="pa")
                for q in range(gn):
                    b = b0 + q
                    tcnt = int(tB[b])
                    for j in range(tcnt):
                        g = posB[b] + j
                        ensure_chunk(gB_tiles, "B", g // p.GT)
                        neigh = gB_tiles[g // p.GT][:, g % p.GT, :]
                        M = mpool.tile([128, 128], bf16, tag="M")
                        if j % 4 == 3:
                            sq = mpool.tile([128, 128], f32, tag="sq")
                            nc.scalar.activation(sq[:], iota_ps[:], AF.Square,
                                                 bias=nlrB_sb[:, g:g + 1])
                            nc.scalar.activation(M[:], sq[:], AF.Relu,
                                                 scale=-1.0, bias=1.0)
                        else:
                            nc.vector.tensor_scalar(
                                out=M[:], in0=iota_ps[:],
                                scalar1=lrB_sb[:, g:g + 1],
                                scalar2=None, op0=mybir.AluOpType.is_equal)
                        nc.tensor.matmul(pa[:, q * 128:(q + 1) * 128],
                                         lhsT=neigh, rhs=M[:],
                                         start=(j == 0), stop=(j == tcnt - 1))
                aggB = aggBpool.tile([D, 512], bf16, tag="aggB")
                nc.vector.tensor_copy(out=aggB[:], in_=pa[:])

                po = po_pool.tile([128, 512], f32, tag="po")
                for q in range(gn):
                    b = b0 + q
                    nb = min(128, NB - b * 128)
                    sl = slice(q * 128, q * 128 + D)
                    nc.tensor.matmul(po[:nb, sl],
                                     lhsT=xT_own[:, b * 128:b * 128 + nb],
                                     rhs=WnW[:], start=True, stop=False)
                    nc.tensor.matmul(po[:nb, sl],
                                     lhsT=aggsA[gi][:, q * 128:q * 128 + nb],
                                     rhs=WgW[:], start=False, stop=False)
                    nc.tensor.matmul(po[:nb, sl],
                                     lhsT=aggB[:, q * 128:q * 128 + nb],
                                     rhs=WgW[:], start=False, stop=False)
                    nc.tensor.matmul(po[:nb, sl], lhsT=onesR[:1, :nb],
                                     rhs=biasR[:1, :], start=False, stop=True)
                for q in range(gn):
                    b = b0 + q
                    nb = min(128, NB - b * 128)
                    if ost is None:
                        ost = ospool.tile([128, OCH * D], f32, tag="ost")
                        ost_base = b
                        ost_n = 0
                    nc.scalar.activation(ost[:nb, ost_n * D:(ost_n + 1) * D],
                                         po[:nb, q * 128:q * 128 + D], AF.Tanh)
                    ost_n += 1
                    if (ost_n == OCH) or (b == NBLK - 1):
                        rows0 = ost_base * 128
                        nfull_o = ost_n if nb == 128 else ost_n - 1
                        if nfull_o:
                            nc.sync.dma_start(
                                out=out_d[rows0:rows0 + nfull_o * 128, :].rearrange(
                                    "(t p) d -> p t d", p=128),
                                in_=ost[:, :nfull_o * D].rearrange(
                                    "p (t d) -> p t d", d=D))
                        if nb != 128:
                            nc.sync.dma_start(
                                out=out_d[rows0 + nfull_o * 128:
                                          rows0 + nfull_o * 128 + nb, :],
                                in_=ost[:nb, nfull_o * D:nfull_o * D + D])
                        ost = None
                b0 += gn

    nc.compile()
    return nc


# ---------------------------------------------------------------- host entry

def _host_prep(p: P, x, edge_index, Wn_w, Wn_b, Wg_w, Wg_b, Wa_w, Wa_b):
    x = np.asarray(x, np.float32)
    xT = np.ascontiguousarray(x.T).astype(BF16)
    tA, tB, LA, LB, LAg, LBg, per_core = prep_edges(
        p, np.asarray(edge_index)[0], np.asarray(edge_index)[1])

    shared = {
        "xT": xT,
        "WaW": np.asarray(Wa_w, np.float32).astype(BF16),
        "WaB": np.asarray(Wa_b, np.float32).reshape(p.D, 1),
        "WnW": np.asarray(Wn_w, np.float32).astype(BF16),
        "WgW": np.asarray(Wg_w, np.float32).astype(BF16),
        "biasR": (np.asarray(Wn_b, np.float32)
                  + np.asarray(Wg_b, np.float32)).reshape(1, p.D).astype(BF16),
        "onesR": np.ones((1, p.D), BF16),
        "iota": np.tile(np.arange(p.D, dtype=np.float32)[:, None],
                        (1, p.D)).astype(BF16),
        "ident": np.eye(p.D, dtype=np.float32).astype(BF16),
    }
    in_maps = []
    for k in range(p.NCORES):
        m = dict(shared)
        m["xT_own"] = np.ascontiguousarray(xT[:, k * p.NB:(k + 1) * p.NB])
        pc = per_core[k]
        m["idxA"], m["idxB"] = pc["idxA"], pc["idxB"]
        m["lrA"], m["lrB"] = pc["lrA"], pc["lrB"]
        m["nlrA"], m["nlrB"] = -pc["lrA"], -pc["lrB"]
        in_maps.append(m)
    return tA, tB, LA, LB, LAg, LBg, in_maps


TRACE = False      # set True (e.g. from test.py) to capture an NTFF profile
LAST = None        # last BassKernelResults, for profiling/inspection


def kernel(**inputs) -> np.ndarray:
    global LAST
    from concourse import bass_utils
    bass_utils.upload_artifacts = lambda tmpdir: "local://" + tmpdir

    p = P()
    tA, tB, LA, LB, LAg, LBg, in_maps = _host_prep(p, **inputs)
    nc = build(p, tA, tB, LA, LB, LAg, LBg)
    kw = dict(trace=True, trace_cores=list(range(p.NCORES))) if TRACE else {}
    res = bass_utils.run_bass_kernel_spmd(
        nc, in_maps, core_ids=list(range(p.NCORES)), **kw)
    LAST = res
    out = np.concatenate([res.results[k]["out"] for k in range(p.NCORES)], axis=0)
    return out.astype(np.float32)
